# revision 3
# baseline (speedup 1.0000x reference)
"""MultiHeadSemGConv Trainium2 kernel.

Computes, for x:[B,N,CIN], W:[H,2,CIN,HC], e:[H,N*K], bias:[H,HC],
rows/cols:[N*K] (int32 edge list):

    h = einsum('bnc,hscd->shbnd', x, W)             # two projections per head
    A = softmax(scatter(e at (rows,cols), NEG))     # [H,N,N]
    out[h,b] = diag(A)*h0 + (A - diag)@h1 + bias    # -> [B,N,H*HC]

Strategy: pure data-parallel over batch across 8 NeuronCores.  The tiny
[H,98,98] adjacency softmax is precomputed on host (it is O(40K) elements);
the heavy lifting (x projection + graph mixing over 100MB of activations)
runs on device:

  per core (128 samples):
    - DMA x in flat 128-partition tiles, casting f32->fp16 in the DMA
    - PE transpose (matmul with identity) -> xT [c(2x128 chunks), bn] in SBUF
    - phase 1, per sample b: h[98,512] = xT[:, 98b:98b+128].T @ Wall
      (2 accumulating fp16 matmuls, f32 PSUM)
    - phase 2, per 8-sample group, per head: 2 accumulating matmuls with
      host-built graph matrices (diag-embed & A_off^T, zero-padded to K=128);
      bias added during the PSUM->SBUF copy on DVE
    - DMA out f32
"""

import os
import sys

import numpy as np

try:
    import concourse.bass as bass  # noqa: F401
except Exception:  # pragma: no cover - fresh grading dir fallback
    for p in ("/opt/trn_rl_repo", "/root/.axon_site/_ro/trn_rl_repo"):
        if os.path.isdir(p) and p not in sys.path:
            sys.path.insert(0, p)
    import concourse.bass as bass  # noqa: F401

# ---------------------------------------------------------------- constants
NLM = 98          # landmarks (graph nodes)
HEADS = 4
CIN = 256
HC = 64
HD = 512          # h width = 2 (s) * 4 (heads) * 64 (d)
B = 1024
NCORES = 8
NS = B // NCORES  # samples per core = 128
P = 128
G = 8             # samples per output group
NGRP = NS // G    # 16 groups per core
HALF_S = NS // 2  # 64 samples per half
NFT_H = HALF_S * NLM // P   # flat 128-row tiles per half = 49
HALF_C = HALF_S * NLM       # xT columns per half = 6272
OVL = 30                    # overlap cols so every phase-1 lhsT can be m=128
DGRP = 7                    # flat tiles per input DMA (7 DMA groups per half)
NEG = -9e15

_CACHE = {}


def _build_nc():
    import concourse.mybir as mybir
    import concourse.tile as tile
    from concourse import bacc

    f16 = mybir.dt.float16
    f32 = mybir.dt.float32

    nc = bacc.Bacc(None, target_bir_lowering=False)

    x = nc.dram_tensor("x", [NS * NLM, CIN], f32, kind="ExternalInput")
    wall = nc.dram_tensor("wall", [P, 2, HD], f16, kind="ExternalInput")
    gmat = nc.dram_tensor("gmat", [P, 2 * HEADS * NLM], f16, kind="ExternalInput")
    biast = nc.dram_tensor("biast", [NLM, G * 256], f32, kind="ExternalInput")
    ident = nc.dram_tensor("ident", [P, P], f16, kind="ExternalInput")
    out = nc.dram_tensor("out", [NS * NLM, CIN], f32, kind="ExternalOutput")

    with tile.TileContext(nc) as tc:
        with (
            tc.tile_pool(name="const", bufs=1) as constp,
            tc.tile_pool(name="xin", bufs=3) as xinp,
            tc.tile_pool(name="xt", bufs=1) as xtp,
            tc.tile_pool(name="hgrp", bufs=2) as hgp,
            tc.tile_pool(name="osb", bufs=2) as osbp,
            tc.tile_pool(name="ptr", bufs=2, space="PSUM") as ptrp,
            tc.tile_pool(name="phs", bufs=2, space="PSUM") as phsp,
            tc.tile_pool(name="pout", bufs=4, space="PSUM") as poutp,
        ):
            ident_sb = constp.tile([P, P], f16, tag="ident")
            nc.sync.dma_start(ident_sb[:], ident[:])
            wall_sb = constp.tile([P, 2, HD], f16, tag="wall")
            nc.sync.dma_start(wall_sb[:], wall[:])
            gm_sb = constp.tile([P, 2 * HEADS * NLM], f16, tag="gmat")
            nc.sync.dma_start(gm_sb[:], gmat[:])
            bias_sb = constp.tile([NLM, G * 256], f32, tag="biast")
            nc.sync.dma_start(bias_sb[:], biast[:])

            xt_half = [
                xtp.tile([P, 2, HALF_C + OVL], f16, tag=f"xt{h}", name=f"xt{h}")
                for h in range(2)
            ]
            # pad cols of half 1 (read by the very last sample's m=128 lhsT)
            nc.vector.memset(xt_half[1][:, :, HALF_C:], 0.0)

            bias3 = bias_sb[:].rearrange("p (s c) -> p s c", s=G)

            def emit_a_group(hf, g7):
                """DMA DGRP flat x tiles (cast to fp16) and transpose them
                into xt_half[hf] columns."""
                xin = xinp.tile([P, DGRP, CIN], f16, tag="xin")
                base = (hf * NFT_H + g7 * DGRP) * P
                nc.gpsimd.dma_start(
                    xin[:],
                    x[base : base + DGRP * P, :].rearrange("(t p) c -> p t c", p=P),
                )
                for t in range(DGRP):
                    ptr = ptrp.tile([P, 2, P], mybir.dt.float32, tag="ptr")
                    for cc in range(2):
                        nc.tensor.matmul(
                            ptr[:, cc, :],
                            xin[:, t, cc * P : (cc + 1) * P],
                            ident_sb[:],
                            start=True,
                            stop=True,
                        )
                    ftl = g7 * DGRP + t
                    nc.scalar.copy(
                        out=xt_half[hf][:, :, ftl * P : (ftl + 1) * P], in_=ptr[:]
                    )
                    if hf == 1 and ftl == 0:
                        # first OVL cols of half1 also complete half0's tail
                        nc.scalar.copy(
                            out=xt_half[0][:, :, HALF_C:], in_=ptr[:, :, :OVL]
                        )

            def emit_b_group(gi):
                """Phase 1 for G samples, then phase 2 + bias + store."""
                hgrp = hgp.tile([P, G * HD], f16, tag="hgrp")
                hf = (gi * G) // HALF_S
                for si in range(G):
                    b = gi * G + si
                    lb = b - hf * HALF_S
                    hps = phsp.tile([P, HD], mybir.dt.float32, tag="hps")
                    for cc in range(2):
                        nc.tensor.matmul(
                            hps[:],
                            xt_half[hf][:, cc, NLM * lb : NLM * lb + P],
                            wall_sb[:, cc, :],
                            start=(cc == 0),
                            stop=(cc == 1),
                        )
                    dst = hgrp[:, si * HD : (si + 1) * HD]
                    if si % 2 == 0:
                        nc.vector.tensor_copy(dst, hps[:])
                    else:
                        nc.scalar.copy(out=dst, in_=hps[:])

                hg3 = hgrp[:].rearrange("p (s f) -> p s f", s=G)
                osb = osbp.tile([NLM, G * 256], mybir.dt.float32, tag="osb")
                osb3 = osb[:].rearrange("p (s c) -> p s c", s=G)
                for hd in range(HEADS):
                    pouts = poutp.tile([NLM, G * HC], mybir.dt.float32, tag="pout")
                    po3 = pouts[:].rearrange("p (s f) -> p s f", s=G)
                    for prt in range(2):
                        q = hd * 2 + prt
                        nc.tensor.matmul(
                            po3,
                            gm_sb[:, q * NLM : (q + 1) * NLM],
                            hg3[:, :, prt * 256 + hd * HC : prt * 256 + (hd + 1) * HC],
                            start=(prt == 0),
                            stop=(prt == 1),
                        )
                    nc.vector.tensor_add(
                        out=osb3[:, :, hd * HC : (hd + 1) * HC],
                        in0=po3,
                        in1=bias3[:, :, hd * HC : (hd + 1) * HC],
                    )
                nc.sync.dma_start(
                    out[gi * G * NLM : (gi + 1) * G * NLM, :].rearrange(
                        "(s i) c -> i s c", s=G
                    ),
                    osb3,
                )

            # half 0 inputs + transposes
            for g7 in range(DGRP):
                emit_a_group(0, g7)
            # first group of half 1 early: provides half0's OVL tail columns
            emit_a_group(1, 0)
            # half 0 compute, interleaved with remaining half-1 transposes
            for gi in range(NGRP // 2):
                emit_b_group(gi)
                if 1 + gi < DGRP:
                    emit_a_group(1, 1 + gi)
            for gi in range(NGRP // 2, NGRP):
                emit_b_group(gi)

    nc.compile()
    return nc


def _host_prep(W, e, bias, rows, cols):
    """Precompute fp16 device constants from the small parameter tensors."""
    W = np.asarray(W, np.float32)
    e = np.asarray(e, np.float32)
    bias = np.asarray(bias, np.float32)
    rows = np.asarray(rows, np.int64)
    cols = np.asarray(cols, np.int64)

    logits = np.full((HEADS, NLM, NLM), NEG, np.float64)
    logits[:, rows, cols] = e.astype(np.float64)
    m = logits.max(axis=-1, keepdims=True)
    p = np.exp(logits - m)
    A = p / p.sum(axis=-1, keepdims=True)            # [H, N, N]
    dg = np.einsum("hii->hi", A).copy()              # [H, N]
    A_off = A.copy()
    np.einsum("hii->hi", A_off)[:] = 0.0

    # Wall: [c, (s, h, d)] -> chunked [128, 2, 512]
    wr = W.transpose(2, 1, 0, 3).reshape(CIN, 2 * HEADS * HC)   # [c, shd]
    wall = np.ascontiguousarray(
        wr.reshape(2, P, 2 * HEADS * HC).transpose(1, 0, 2)
    ).astype(np.float16)

    # graph matrices, zero-padded to K=128 rows: [j, (head, part, i)]
    gm = np.zeros((P, HEADS, 2, NLM), np.float32)
    idx = np.arange(NLM)
    for h in range(HEADS):
        gm[idx, h, 0, idx] = dg[h]
        gm[:NLM, h, 1, :] = A_off[h].T
    gmat = np.ascontiguousarray(gm.reshape(P, 2 * HEADS * NLM)).astype(np.float16)

    bcat = bias.reshape(HEADS * HC)                  # col = h*64+d
    biast = np.ascontiguousarray(np.tile(bcat, (NLM, G))).astype(np.float32)

    ident = np.eye(P, dtype=np.float16)
    return {"wall": wall, "gmat": gmat, "biast": biast, "ident": ident}


def kernel(x, W, e, bias, rows, cols):
    from concourse.bass_utils import run_bass_kernel_spmd

    if "nc" not in _CACHE:
        _CACHE["nc"] = _build_nc()
    nc = _CACHE["nc"]

    consts = _host_prep(W, e, bias, rows, cols)
    x = np.ascontiguousarray(np.asarray(x, np.float32)).reshape(B, NLM, CIN)

    in_maps = []
    for ci in range(NCORES):
        shard = np.ascontiguousarray(
            x[ci * NS : (ci + 1) * NS].reshape(NS * NLM, CIN)
        )
        in_maps.append({"x": shard, **consts})

    res = run_bass_kernel_spmd(
        nc,
        in_maps,
        core_ids=list(range(NCORES)),
        trace=bool(int(os.environ.get("KERNEL_TRACE", "0"))),
    )
    _CACHE["last_results"] = res

    out = np.concatenate(
        [r["out"].reshape(NS, NLM, HEADS * HC) for r in res.results], axis=0
    )
    return out


# revision 5
# speedup vs baseline: 1.1597x; 1.1597x over previous
"""MultiHeadSemGConv Trainium2 kernel.

Computes, for x:[B,N,CIN], W:[H,2,CIN,HC], e:[H,N*K], bias:[H,HC],
rows/cols:[N*K] (int32 edge list):

    h = einsum('bnc,hscd->shbnd', x, W)             # two projections per head
    A = softmax(scatter(e at (rows,cols), NEG))     # [H,N,N]
    out[h,b] = diag(A)*h0 + (A - diag)@h1 + bias    # -> [B,N,H*HC]

Strategy: pure data-parallel over batch across 8 NeuronCores.  The tiny
[H,98,98] adjacency softmax is precomputed on host; the heavy lifting
(x projection + graph mixing over 100MB of activations) runs on device:

  per core (128 samples):
    - DMA x in flat 128-partition tiles, casting f32->fp16 in the DMA
    - PE transpose (matmul with identity) -> xT chunk tiles
      [c(2x128), 16*98+30 cols] fp16 in SBUF (30-col overlap keeps every
      per-sample phase-1 stationary at m=128)
    - phase 1, per sample b: h[128,512] = xT[:, 98b:98b+128].T @ Wall
      (2 accumulating fp16 matmuls, f32 PSUM), 2 samples per PSUM tile
    - phase 2, per 8-sample group, per head: 2 accumulating matmuls with
      host-built graph matrices (diag-embed & A_off^T, zero-padded to
      K=128); bias added during the PSUM->SBUF copy on DVE
    - DMA out f32
"""

import os
import sys

import numpy as np

try:
    import concourse.bass as bass  # noqa: F401
except Exception:  # pragma: no cover - fresh grading dir fallback
    for p in ("/opt/trn_rl_repo", "/root/.axon_site/_ro/trn_rl_repo"):
        if os.path.isdir(p) and p not in sys.path:
            sys.path.insert(0, p)
    import concourse.bass as bass  # noqa: F401

# ---------------------------------------------------------------- constants
NLM = 98          # landmarks (graph nodes)
HEADS = 4
CIN = 256
HC = 64
HD = 512          # h width = 2 (s) * 4 (heads) * 64 (d)
B = 1024
NCORES = 8
NS = B // NCORES  # samples per core = 128
P = 128
G = 8             # samples per output group
NGRP = NS // G    # 16 groups per core
OVL = 30          # overlap cols so every phase-1 lhsT can be m=128
NEG = -9e15

CHS = 16                    # samples per xT chunk
NCH = NS // CHS             # 8 chunks
CHW = CHS * NLM             # 1568 cols per chunk (+OVL)
NFT = NS * NLM // P         # 98 flat 128-row tiles
NPAIR = NFT // 2            # 49 transpose pairs
DGF = 14                    # flat tiles per input DMA group
NDG = NFT // DGF            # 7 DMA groups

_CACHE = {}


def _build_nc():
    import concourse.mybir as mybir
    import concourse.tile as tile
    from concourse import bacc

    f16 = mybir.dt.float16
    f32 = mybir.dt.float32

    nc = bacc.Bacc(None, target_bir_lowering=False)

    x = nc.dram_tensor("x", [NS * NLM, CIN], f32, kind="ExternalInput")
    wall = nc.dram_tensor("wall", [P, 2, HD], f16, kind="ExternalInput")
    gmat = nc.dram_tensor("gmat", [P, 2 * HEADS * NLM], f16, kind="ExternalInput")
    biast = nc.dram_tensor("biast", [NLM, G * 256], f32, kind="ExternalInput")
    ident = nc.dram_tensor("ident", [P, P], f16, kind="ExternalInput")
    out = nc.dram_tensor("out", [NS * NLM, CIN], f32, kind="ExternalOutput")

    with tile.TileContext(nc) as tc:
        with (
            tc.tile_pool(name="const", bufs=1) as constp,
            tc.tile_pool(name="xin", bufs=3) as xinp,
            tc.tile_pool(name="xt", bufs=1) as xtp,
            tc.tile_pool(name="hgrp", bufs=2) as hgp,
            tc.tile_pool(name="osb", bufs=2) as osbp,
            tc.tile_pool(name="ptr", bufs=2, space="PSUM") as ptrp,
            tc.tile_pool(name="phs", bufs=2, space="PSUM") as phsp,
            tc.tile_pool(name="pout", bufs=2, space="PSUM") as poutp,
        ):
            ident_sb = constp.tile([P, P], f16, tag="ident")
            nc.sync.dma_start(ident_sb[:], ident[:])
            wall_sb = constp.tile([P, 2, HD], f16, tag="wall")
            nc.sync.dma_start(wall_sb[:], wall[:])
            gm_sb = constp.tile([P, 2 * HEADS * NLM], f16, tag="gmat")
            nc.sync.dma_start(gm_sb[:], gmat[:])
            bias_sb = constp.tile([NLM, G * 256], f32, tag="biast")
            nc.sync.dma_start(bias_sb[:], biast[:])

            xt = [
                xtp.tile([P, 2, CHW + OVL], f16, tag=f"xt{k}", name=f"xt{k}")
                for k in range(NCH)
            ]
            nc.vector.memset(xt[NCH - 1][:, :, CHW:], 0.0)

            bias3 = bias_sb[:].rearrange("p (s c) -> p s c", s=G)

            def route_piece(g0, ptr, off, w):
                """Copy ptr[:, :, off:off+w] (global xT cols [g0,g0+w)) into
                the chunk tiles, including overlap duplication."""
                while w > 0:
                    k = g0 // CHW
                    lo = g0 - k * CHW
                    pw = min(w, CHW - lo)
                    nc.scalar.copy(
                        out=xt[k][:, :, lo : lo + pw],
                        in_=ptr[:, :, off : off + pw],
                    )
                    # overlap region of the previous chunk
                    if k > 0 and lo < OVL:
                        ow = min(pw, OVL - lo)
                        nc.scalar.copy(
                            out=xt[k - 1][:, :, CHW + lo : CHW + lo + ow],
                            in_=ptr[:, :, off : off + ow],
                        )
                    g0 += pw
                    off += pw
                    w -= pw

            def emit_a_group(dg):
                """DMA DGF flat x tiles (cast to fp16) and transpose them."""
                xin = xinp.tile([P, DGF, CIN], f16, tag="xin")
                base = dg * DGF * P
                nc.gpsimd.dma_start(
                    xin[:],
                    x[base : base + DGF * P, :].rearrange("(t p) c -> p t c", p=P),
                )
                for pr in range(DGF // 2):
                    ptr = ptrp.tile([P, 2, 2 * P], mybir.dt.float32, tag="ptr")
                    for a in range(2):
                        t = pr * 2 + a
                        for cc in range(2):
                            nc.tensor.matmul(
                                ptr[:, cc, a * P : (a + 1) * P],
                                xin[:, t, cc * P : (cc + 1) * P],
                                ident_sb[:],
                                start=True,
                                stop=True,
                            )
                    route_piece((dg * DGF + pr * 2) * P, ptr, 0, 2 * P)

            def emit_b_group(gi):
                """Phase 1 for G samples, then phase 2 + bias + store."""
                hgrp = hgp.tile([P, G * HD], f16, tag="hgrp")
                ck = (gi * G) // CHS
                for pi in range(G // 2):
                    hps = phsp.tile([P, 2, HD], mybir.dt.float32, tag="hps")
                    for a in range(2):
                        b = gi * G + pi * 2 + a
                        lb = b - ck * CHS
                        for cc in range(2):
                            nc.tensor.matmul(
                                hps[:, a, :],
                                xt[ck][:, cc, NLM * lb : NLM * lb + P],
                                wall_sb[:, cc, :],
                                start=(cc == 0),
                                stop=(cc == 1),
                            )
                    dst = hgrp[:, pi * 2 * HD : (pi + 1) * 2 * HD].rearrange(
                        "p (a f) -> p a f", a=2
                    )
                    if (gi * G // 2 + pi) % 5 < 3:
                        nc.scalar.copy(out=dst, in_=hps[:])
                    else:
                        nc.vector.tensor_copy(dst, hps[:])

                hg3 = hgrp[:].rearrange("p (s f) -> p s f", s=G)
                osb = osbp.tile([NLM, G * 256], mybir.dt.float32, tag="osb")
                osb3 = osb[:].rearrange("p (s c) -> p s c", s=G)
                for hd in range(HEADS):
                    pouts = poutp.tile([NLM, G * HC], mybir.dt.float32, tag="pout")
                    po3 = pouts[:].rearrange("p (s f) -> p s f", s=G)
                    for prt in range(2):
                        q = hd * 2 + prt
                        nc.tensor.matmul(
                            po3,
                            gm_sb[:, q * NLM : (q + 1) * NLM],
                            hg3[:, :, prt * 256 + hd * HC : prt * 256 + (hd + 1) * HC],
                            start=(prt == 0),
                            stop=(prt == 1),
                        )
                    nc.vector.tensor_add(
                        out=osb3[:, :, hd * HC : (hd + 1) * HC],
                        in0=po3,
                        in1=bias3[:, :, hd * HC : (hd + 1) * HC],
                    )
                nc.sync.dma_start(
                    out[gi * G * NLM : (gi + 1) * G * NLM, :].rearrange(
                        "(s i) c -> i s c", s=G
                    ),
                    osb3,
                )

            # chunk k is fully transposed once DMA group ceil((1598+1568k)/1792)
            # has been processed; interleave A and B so PE never starves.
            ready_dg = [
                -(-(CHW * k + CHW + OVL) // (DGF * P)) for k in range(NCH)
            ]  # per chunk, 1-indexed count of A groups needed
            ready_dg[NCH - 1] = NDG
            emitted = 0
            for gi in range(NGRP):
                need = ready_dg[(gi * G) // CHS]
                while emitted < need:
                    emit_a_group(emitted)
                    emitted += 1
                emit_b_group(gi)

    nc.compile()
    return nc


def _host_prep(W, e, bias, rows, cols):
    """Precompute fp16 device constants from the small parameter tensors."""
    W = np.asarray(W, np.float32)
    e = np.asarray(e, np.float32)
    bias = np.asarray(bias, np.float32)
    rows = np.asarray(rows, np.int64)
    cols = np.asarray(cols, np.int64)

    logits = np.full((HEADS, NLM, NLM), NEG, np.float64)
    logits[:, rows, cols] = e.astype(np.float64)
    m = logits.max(axis=-1, keepdims=True)
    p = np.exp(logits - m)
    A = p / p.sum(axis=-1, keepdims=True)            # [H, N, N]
    dg = np.einsum("hii->hi", A).copy()              # [H, N]
    A_off = A.copy()
    np.einsum("hii->hi", A_off)[:] = 0.0

    # Wall: [c, (s, h, d)] -> chunked [128, 2, 512]
    wr = W.transpose(2, 1, 0, 3).reshape(CIN, 2 * HEADS * HC)   # [c, shd]
    wall = np.ascontiguousarray(
        wr.reshape(2, P, 2 * HEADS * HC).transpose(1, 0, 2)
    ).astype(np.float16)

    # graph matrices, zero-padded to K=128 rows: [j, (head, part, i)]
    gm = np.zeros((P, HEADS, 2, NLM), np.float32)
    idx = np.arange(NLM)
    for h in range(HEADS):
        gm[idx, h, 0, idx] = dg[h]
        gm[:NLM, h, 1, :] = A_off[h].T
    gmat = np.ascontiguousarray(gm.reshape(P, 2 * HEADS * NLM)).astype(np.float16)

    bcat = bias.reshape(HEADS * HC)                  # col = h*64+d
    biast = np.ascontiguousarray(np.tile(bcat, (NLM, G))).astype(np.float32)

    ident = np.eye(P, dtype=np.float16)
    return {"wall": wall, "gmat": gmat, "biast": biast, "ident": ident}


def kernel(x, W, e, bias, rows, cols):
    from concourse.bass_utils import run_bass_kernel_spmd

    if "nc" not in _CACHE:
        _CACHE["nc"] = _build_nc()
    nc = _CACHE["nc"]

    consts = _host_prep(W, e, bias, rows, cols)
    x = np.ascontiguousarray(np.asarray(x, np.float32)).reshape(B, NLM, CIN)

    in_maps = []
    for ci in range(NCORES):
        shard = np.ascontiguousarray(
            x[ci * NS : (ci + 1) * NS].reshape(NS * NLM, CIN)
        )
        in_maps.append({"x": shard, **consts})

    res = run_bass_kernel_spmd(
        nc,
        in_maps,
        core_ids=list(range(NCORES)),
        trace=bool(int(os.environ.get("KERNEL_TRACE", "0"))),
    )
    _CACHE["last_results"] = res

    out = np.concatenate(
        [r["out"].reshape(NS, NLM, HEADS * HC) for r in res.results], axis=0
    )
    return out
